# revision 26
# baseline (speedup 1.0000x reference)
"""LoRA generator kernel for Trainium2, sharded over 8 NeuronCores by layer.

Reference computation:
  pe = (condition @ W_proj + b_proj)                        (B=2, 224, 512)
  A  = (gelu(pe@WA1+bA1) @ WA2 + bA2) -> (B, L, 7, 16, 64)
  Bm = (gelu(pe@WB1+bB1) @ WB2 + bB2) -> (B, L, 7, 64, 16)
  out per (b, layer): concat over t of [tile_cols(A)*scA (16 x in_d),
                                        tile_rows(B)*scB (out_d x 16)]

Each core handles 4 layers (28 of the 224 projections). All HBM traffic is
bf16 (W_proj cast on host, output upcast on host), halving the dominant DMA
bytes. Output pieces are written with 16KB descriptors: the decoder outputs
are scattered (one small SBUF->SBUF DMA per piece family) into wide tiles
whose partition rows hold the piece content base, then the periodic tiling
is materialized in-place by log2 doubling copies on the Vector/Scalar
engines, and each piece row streams out as contiguous 16KB reads.
"""
import sys

sys.path.insert(0, "/opt/trn_rl_repo")

import numpy as np
import ml_dtypes

import concourse.bass as bass
import concourse.bacc as bacc
import concourse.mybir as mybir
import concourse.tile as tile
from concourse.bass_utils import run_bass_kernel_spmd

F32 = mybir.dt.float32
BF16 = mybir.dt.bfloat16
NPBF16 = ml_dtypes.bfloat16

NCORES = 8
NUM_LAYERS = 32
RANK = 16
PED = 512
EMB = 384
T = 7
L = NUM_LAYERS // NCORES          # 4 layers per core
LT = L * T                        # 28 projections per core
RPL = 2 * T                       # 14 rows per layer; row = t*2 + b
CHUNK = T * PED                   # 3584 W_proj cols per layer

IN_DS = [4096, 4096, 4096, 4096, 4096, 4096, 11008]
OUT_DS = [4096, 1024, 1024, 4096, 11008, 11008, 4096]
A_SIZES = [16 * d for d in IN_DS]
B_SIZES = [16 * d for d in OUT_DS]
LAYER_SIZE = sum(A_SIZES) + sum(B_SIZES)   # 1150976
OFF_A = []
OFF_B = []
_o = 0
for _t in range(T):
    OFF_A.append(_o)
    _o += A_SIZES[_t]
    OFF_B.append(_o)
    _o += B_SIZES[_t]
OUT_SZ = 2 * L * LAYER_SIZE
LLS = L * LAYER_SIZE               # out stride between b=0 and b=1
WV = 8192                          # A/B tile row width (16KB bf16)

# CoreSim has no Gelu; debug harnesses set this to validate dataflow
SIM_GELU_IDENTITY = False

# Per-layer projections are processed in "stage order" s = rt(t) so that the
# t=6 (down_proj) rows land on partitions 0 and 7: its rank-scatter AP then
# has a stride-7 partition dim with zero offset, which the dependency
# tracker handles (k*pitch stride + nonzero partition offset does not).
RT = [1, 2, 3, 4, 5, 6, 0]         # stage slot of projection t
TOF = [6, 0, 1, 2, 3, 4, 5]        # projection t in stage slot s


def _build_nc():
    nc = bacc.Bacc(None, target_bir_lowering=False, debug=False)

    cond = nc.declare_dram_parameter("cond", [128, 6], BF16, isOutput=False)
    wp = nc.declare_dram_parameter("wp", [EMB, LT * PED], BF16, isOutput=False)
    bias2 = nc.declare_dram_parameter("bias2", [2, L * CHUNK], BF16, isOutput=False)
    wa1 = nc.declare_dram_parameter("wa1", [128, 1024], BF16, isOutput=False)
    wb1 = nc.declare_dram_parameter("wb1", [128, 1024], BF16, isOutput=False)
    wa2 = nc.declare_dram_parameter("wa2", [128, 2048], BF16, isOutput=False)
    wb2 = nc.declare_dram_parameter("wb2", [128, 2048], BF16, isOutput=False)
    ba1 = nc.declare_dram_parameter("ba1", [128, 2], F32, isOutput=False)
    bb1 = nc.declare_dram_parameter("bb1", [128, 2], F32, isOutput=False)
    sca = nc.declare_dram_parameter("sca", [128, 2 * LT], BF16, isOutput=False)
    scb = nc.declare_dram_parameter("scb", [128, 2 * LT], BF16, isOutput=False)
    sba2 = nc.declare_dram_parameter("sba2", [16, L * 1024], BF16, isOutput=False)
    sbb2 = nc.declare_dram_parameter("sbb2", [16, L * 1024], BF16, isOutput=False)
    ident = nc.declare_dram_parameter("ident", [16, 16], BF16, isOutput=False)
    out = nc.declare_dram_parameter("out", [OUT_SZ], BF16, isOutput=True)

    with tile.TileContext(nc) as tc:
        with (
            tc.tile_pool(name="const", bufs=1) as cpool,
            tc.tile_pool(name="wp", bufs=2) as wpool,
            tc.tile_pool(name="pe", bufs=2) as pepool,
            tc.tile_pool(name="dec", bufs=2) as decpool,
            tc.tile_pool(name="abuf", bufs=2) as apool,
            tc.tile_pool(name="bbuf", bufs=2) as bpool,
            tc.tile_pool(name="ps", bufs=1, space="PSUM") as ps,
        ):
            cond_sb = cpool.tile([128, 6], BF16)
            nc.sync.dma_start(cond_sb[:], cond[:])
            bias2_sb = cpool.tile([2, L * CHUNK], BF16)
            nc.sync.dma_start(bias2_sb[:], bias2[:])
            wa1_sb = cpool.tile([128, 1024], BF16)
            nc.sync.dma_start(wa1_sb[:], wa1[:])
            wb1_sb = cpool.tile([128, 1024], BF16)
            nc.scalar.dma_start(wb1_sb[:], wb1[:])
            wa2_sb = cpool.tile([128, 2048], BF16)
            nc.sync.dma_start(wa2_sb[:], wa2[:])
            wb2_sb = cpool.tile([128, 2048], BF16)
            nc.scalar.dma_start(wb2_sb[:], wb2[:])
            ba1_sb = cpool.tile([128, 2], F32)
            nc.sync.dma_start(ba1_sb[:], ba1[:])
            bb1_sb = cpool.tile([128, 2], F32)
            nc.scalar.dma_start(bb1_sb[:], bb1[:])
            sca_sb = cpool.tile([128, 2 * LT], BF16)
            nc.sync.dma_start(sca_sb[:], sca[:])
            scb_sb = cpool.tile([128, 2 * LT], BF16)
            nc.scalar.dma_start(scb_sb[:], scb[:])
            sba2_sb = cpool.tile([16, L * 1024], BF16)
            nc.sync.dma_start(sba2_sb[:], sba2[:])
            sbb2_sb = cpool.tile([16, L * 1024], BF16)
            nc.scalar.dma_start(sbb2_sb[:], sbb2[:])
            ident_sb = cpool.tile([16, 16], BF16)
            nc.sync.dma_start(ident_sb[:], ident[:])

            def load_chunk(l):
                tiles = []
                for kc in range(3):
                    t_ = wpool.tile([128, CHUNK], BF16, tag=f"wp{kc}", name=f"wp{kc}")
                    nc.gpsimd.dma_start(
                        t_[:],
                        wp[kc * 128 : (kc + 1) * 128, l * CHUNK : (l + 1) * CHUNK],
                    )
                    tiles.append(t_)
                return tiles

            def pe_layer(l, wp_t):
                """pe rows for layer l -> peT [128 ped-chunk x4, 14 rows] bf16."""
                pest = pepool.tile([2, CHUNK], BF16, tag="pest", name="pest")
                for ltl in range(T):
                    p2 = ps.tile([2, PED], F32, tag=f"p2{ltl % 2}", name="p2")
                    for kc in range(3):
                        nc.tensor.matmul(
                            p2[:],
                            cond_sb[:, kc * 2 : kc * 2 + 2],
                            wp_t[kc][:, ltl * PED : (ltl + 1) * PED],
                            start=(kc == 0),
                            stop=(kc == 2),
                        )
                    nc.vector.tensor_add(
                        pest[0:2, ltl * PED : (ltl + 1) * PED],
                        p2[:],
                        bias2_sb[0:2, l * CHUNK + ltl * PED : l * CHUNK + (ltl + 1) * PED],
                    )
                # gather rows (b*7+t) onto partitions 0..13
                pe2 = pepool.tile([RPL, PED], BF16, tag="pe2", name="pe2")
                p16 = pe2[:, :].ap[0][0]
                pst = pest[:, :].ap[0][0]
                nc.gpsimd.dma_start(
                    bass.AP(pe2[:, :].tensor, 0, [[p16, RPL], [1, PED]]),
                    bass.AP(pest[:, :].tensor, 0, [[pst, 2], [PED, T], [1, PED]]),
                )
                peT = pepool.tile([128, 4 * RPL], BF16, tag="peT", name="peT")
                for mc in range(4):
                    tr = ps.tile([128, RPL], BF16, tag=f"tr{mc % 2}", name="tr")
                    nc.tensor.transpose(
                        tr[:],
                        pe2[:, mc * 128 : (mc + 1) * 128],
                        ident_sb[0:RPL, 0:RPL],
                    )
                    nc.vector.tensor_copy(peT[:, mc * RPL : (mc + 1) * RPL], tr[:])
                return peT

            def decode(l, peT, dec):
                """Decoder MLP for layer l -> o [14 rows, 1024] bf16 (scaled+bias)."""
                w1_sb, b1_sb, sc_sb, w2_sb, sb2_sb = (
                    (wa1_sb, ba1_sb, sca_sb, wa2_sb, sba2_sb)
                    if dec == 0
                    else (wb1_sb, bb1_sb, scb_sb, wb2_sb, sbb2_sb)
                )
                h_sb = []
                for mc in range(2):
                    hp = ps.tile([128, RPL], F32, tag=f"h{mc}", name="hp")
                    for kc in range(4):
                        nc.tensor.matmul(
                            hp[:],
                            w1_sb[:, kc * 256 + mc * 128 : kc * 256 + (mc + 1) * 128],
                            peT[:, kc * RPL : (kc + 1) * RPL],
                            start=(kc == 0),
                            stop=(kc == 3),
                        )
                    hs = decpool.tile(
                        [128, RPL], BF16, tag=f"hs{dec}{mc}", name="hs"
                    )
                    act = (
                        mybir.ActivationFunctionType.Identity
                        if SIM_GELU_IDENTITY
                        else mybir.ActivationFunctionType.Gelu
                    )
                    nc.scalar.activation(
                        hs[:], hp[:], act, bias=b1_sb[:, mc : mc + 1]
                    )
                    nc.vector.tensor_mul(
                        hs[:], hs[:], sc_sb[:, l * RPL : (l + 1) * RPL]
                    )
                    h_sb.append(hs)
                o_sb = decpool.tile([RPL, 1024], BF16, tag=f"o{dec}", name="o")
                for nh in range(2):
                    op = ps.tile([RPL, 512], F32, tag=f"o{nh}", name="op")
                    for kc in range(2):
                        nc.tensor.matmul(
                            op[:],
                            h_sb[kc][:],
                            w2_sb[:, kc * 1024 + nh * 512 : kc * 1024 + (nh + 1) * 512],
                            start=(kc == 0),
                            stop=(kc == 1),
                        )
                    nc.vector.tensor_add(
                        o_sb[:, nh * 512 : (nh + 1) * 512],
                        op[:],
                        sb2_sb[0:RPL, l * 1024 + nh * 512 : l * 1024 + (nh + 1) * 512],
                    )
                return o_sb

            def emit_layer(l, peT):
                oa = decode(l, peT, 0)
                ob = decode(l, peT, 1)

                atile = apool.tile([128, WV], BF16, tag="atile", name="atile")
                btile = bpool.tile([112, WV], BF16, tag="btile", name="btile")
                pa = oa[:, :].ap[0][0]
                pb = ob[:, :].ap[0][0]
                pax = atile[:, :].ap[0][0]
                pbx = btile[:, :].ap[0][0]
                oa_t = oa[:, :].tensor
                ob_t = ob[:, :].tensor
                atile_t = atile[:, :].tensor
                btile_t = btile[:, :].tensor

                # --- seed scatters (SBUF->SBUF) ---
                # rows are b-major stage order: row = b*7 + RT[t]
                # A t<6 (rows 1..6): partition 56b+8t+rp gets rank 2rp at col 0,
                # rank 2rp+1 at col 4096
                for b in range(2):
                    for half in range(2):
                        nc.gpsimd.dma_start(
                            bass.AP(
                                atile_t,
                                48 * b * pax + half * 4096,
                                [[pax, 48], [1, 64]],
                            ),
                            bass.AP(
                                oa_t,
                                (7 * b + 1) * pa + half * 64,
                                [[pa, 6], [128, 8], [1, 64]],
                            ),
                        )
                # A t=6 (rows 0, 7): partition 96+16b+r gets rank r at 0 and 4096
                for half in range(2):
                    nc.gpsimd.dma_start(
                        bass.AP(atile_t, 96 * pax + half * 4096, [[pax, 32], [1, 64]]),
                        bass.AP(oa_t, 0, [[7 * pa, 2], [64, 16], [1, 64]]),
                    )
                # B: partition 56b+8t+copy gets the 1024-elem base block
                nc.gpsimd.dma_start(
                    bass.AP(btile_t, 0, [[pbx, 112], [1, 1024]]),
                    bass.AP(ob_t, 0, [[pb, 14], [0, 8], [1, 1024]]),
                )

                # --- widen by doubling (A on vector, B on scalar) ---
                w = 64
                while w < 4096:
                    nc.vector.tensor_copy(atile[:, w : 2 * w], atile[:, 0:w])
                    nc.vector.tensor_copy(
                        atile[:, 4096 + w : 4096 + 2 * w], atile[:, 4096 : 4096 + w]
                    )
                    w *= 2
                w = 1024
                while w < WV:
                    nc.scalar.copy(btile[:, w : 2 * w], btile[:, 0:w])
                    w *= 2

                # --- output writes ---
                # Descriptors are assigned to the 16 SDMA engines by index
                # within each dma_start, so bulk writes emit multiples of 16
                # descriptors (8KB half-rows via explicit 4096-elem chunking).
                base = l * LAYER_SIZE
                stA45 = A_SIZES[4] + B_SIZES[4]   # A t4 -> t5 stride (241664)
                stB45 = B_SIZES[4] + A_SIZES[5]   # B t4 -> t5 stride (241664)
                for b in range(2):
                    bo = base + b * LLS
                    pA = 48 * b * pax
                    pB = 56 * b * pbx
                    # A t0: 16 x 8KB
                    nc.sync.dma_start(
                        bass.AP(out, bo + OFF_A[0], [[4096, 16], [1, 4096]]),
                        bass.AP(atile_t, pA, [[pax, 8], [4096, 2], [1, 4096]]),
                    )
                    # A t1..t3 (uniform stride 81920): 48 x 8KB
                    nc.sync.dma_start(
                        bass.AP(
                            out, bo + OFF_A[1], [[81920, 3], [4096, 16], [1, 4096]]
                        ),
                        bass.AP(
                            atile_t, pA + 8 * pax, [[pax, 24], [4096, 2], [1, 4096]]
                        ),
                    )
                    # A t4, t5: 32 x 8KB
                    nc.sync.dma_start(
                        bass.AP(
                            out, bo + OFF_A[4], [[stA45, 2], [4096, 16], [1, 4096]]
                        ),
                        bass.AP(
                            atile_t, pA + 32 * pax, [[pax, 16], [4096, 2], [1, 4096]]
                        ),
                    )
                    # A t6 main: 11008-elem rows; read the periodic 8192 source
                    # as two 4096 chunks (content period 64 divides 4096): 32 x 8KB
                    nc.sync.dma_start(
                        bass.AP(
                            out, bo + OFF_A[6], [[11008, 16], [4096, 2], [1, 4096]]
                        ),
                        bass.AP(
                            atile_t,
                            (96 + 16 * b) * pax,
                            [[pax, 16], [0, 2], [1, 4096]],
                        ),
                    )
                    # A t6 tail (2816 elems per rank row): 16 x 5.5KB
                    nc.sync.dma_start(
                        bass.AP(
                            out, bo + OFF_A[6] + 8192, [[11008, 16], [1, 2816]]
                        ),
                        bass.AP(atile_t, (96 + 16 * b) * pax, [[pax, 16], [1, 2816]]),
                    )
                    # B piece for projection t sits at partitions 56b + 8*RT[t]
                    # B t0: 16 x 8KB
                    nc.scalar.dma_start(
                        bass.AP(out, bo + OFF_B[0], [[4096, 16], [1, 4096]]),
                        bass.AP(
                            btile_t,
                            pB + 8 * RT[0] * pbx,
                            [[pbx, 8], [4096, 2], [1, 4096]],
                        ),
                    )
                    # B t1, t2: 2 reps each (4 x 8KB)
                    for t in (1, 2):
                        nc.scalar.dma_start(
                            bass.AP(out, bo + OFF_B[t], [[4096, 4], [1, 4096]]),
                            bass.AP(
                                btile_t,
                                pB + 8 * RT[t] * pbx,
                                [[pbx, 2], [4096, 2], [1, 4096]],
                            ),
                        )
                    # B t3: 16 x 8KB
                    nc.scalar.dma_start(
                        bass.AP(out, bo + OFF_B[3], [[4096, 16], [1, 4096]]),
                        bass.AP(
                            btile_t,
                            pB + 8 * RT[3] * pbx,
                            [[pbx, 8], [4096, 2], [1, 4096]],
                        ),
                    )
                    # B t4, t5: 21.5 reps of 8192 = 8 + 8 + 5 + tail 4096.
                    # reps 0..7 read copies 0..7 of t4 then t5 (contiguous 16
                    # partitions at stage slots 5,6), twice (each 32 x 8KB);
                    # mid reps per t; the 4096 tail reads 2048 from copies 0,1
                    # (base block period is 1024, any 2048-prefix is correct)
                    for rep in range(2):
                        nc.sync.dma_start(
                            bass.AP(
                                out,
                                bo + OFF_B[4] + rep * 65536,
                                [[stB45, 2], [4096, 16], [1, 4096]],
                            ),
                            bass.AP(
                                btile_t,
                                pB + 8 * RT[4] * pbx,
                                [[pbx, 16], [4096, 2], [1, 4096]],
                            ),
                        )
                    for t in (4, 5):
                        nc.scalar.dma_start(
                            bass.AP(
                                out, bo + OFF_B[t] + 131072, [[4096, 10], [1, 4096]]
                            ),
                            bass.AP(
                                btile_t,
                                pB + 8 * RT[t] * pbx,
                                [[pbx, 5], [4096, 2], [1, 4096]],
                            ),
                        )
                        nc.gpsimd.dma_start(
                            bass.AP(
                                out, bo + OFF_B[t] + 172032, [[2048, 2], [1, 2048]]
                            ),
                            bass.AP(
                                btile_t, pB + 8 * RT[t] * pbx, [[pbx, 2], [1, 2048]]
                            ),
                        )
                    # B t6 (stage slot 0): 16 x 8KB
                    nc.scalar.dma_start(
                        bass.AP(out, bo + OFF_B[6], [[4096, 16], [1, 4096]]),
                        bass.AP(btile_t, pB, [[pbx, 8], [4096, 2], [1, 4096]]),
                    )

            # ---- main pipeline ----
            chunks = [load_chunk(0), load_chunk(1)]
            for l in range(L):
                peT = pe_layer(l, chunks[l])
                if l + 2 < L:
                    chunks.append(load_chunk(l + 2))
                emit_layer(l, peT)

    nc.finalize()
    return nc


_NC = None


def _get_nc():
    global _NC
    if _NC is None:
        _NC = _build_nc()
    return _NC


def _marshal(inputs):
    """Build the per-core input maps from full inputs."""
    condition = np.asarray(inputs["condition"], np.float32)
    W_proj = np.asarray(inputs["W_proj"], np.float32)
    b_proj = np.asarray(inputs["b_proj"], np.float32)
    WA1 = np.asarray(inputs["WA1"], np.float32)
    bA1 = np.asarray(inputs["bA1"], np.float32)
    WA2 = np.asarray(inputs["WA2"], np.float32)
    bA2 = np.asarray(inputs["bA2"], np.float32)
    WB1 = np.asarray(inputs["WB1"], np.float32)
    bB1 = np.asarray(inputs["bB1"], np.float32)
    WB2 = np.asarray(inputs["WB2"], np.float32)
    bB2 = np.asarray(inputs["bB2"], np.float32)
    scales = np.asarray(inputs["scales"], np.float32)

    cond_arr = np.zeros((128, 6), np.float32)
    for kc in range(3):
        cond_arr[:, kc * 2 : kc * 2 + 2] = condition[:, kc * 128 : (kc + 1) * 128].T
    wa1_arr = np.zeros((128, 1024), np.float32)
    wb1_arr = np.zeros((128, 1024), np.float32)
    for kc in range(4):
        wa1_arr[:, kc * 256 : (kc + 1) * 256] = WA1[kc * 128 : (kc + 1) * 128, :]
        wb1_arr[:, kc * 256 : (kc + 1) * 256] = WB1[kc * 128 : (kc + 1) * 128, :]
    wa2_arr = np.zeros((128, 2048), np.float32)
    wb2_arr = np.zeros((128, 2048), np.float32)
    for kc in range(2):
        wa2_arr[:, kc * 1024 : (kc + 1) * 1024] = WA2[kc * 128 : (kc + 1) * 128, :]
        wb2_arr[:, kc * 1024 : (kc + 1) * 1024] = WB2[kc * 128 : (kc + 1) * 128, :]
    ba1_arr = np.ascontiguousarray(bA1.reshape(2, 128).T)
    bb1_arr = np.ascontiguousarray(bB1.reshape(2, 128).T)
    ident_arr = np.eye(16, dtype=np.float32)

    in_maps = []
    for c in range(NCORES):
        lt0 = c * LT
        # reorder each layer's 7 projections into stage order s (t = TOF[s])
        wp_full = W_proj[:, lt0 * PED : (lt0 + LT) * PED].reshape(EMB, L, T, PED)
        bp_full = b_proj[lt0 * PED : (lt0 + LT) * PED].reshape(L, T, PED)
        wp_c = np.ascontiguousarray(
            wp_full[:, :, TOF, :].reshape(EMB, LT * PED).astype(NPBF16)
        )
        bias2_row = np.ascontiguousarray(bp_full[:, TOF, :].reshape(L * CHUNK))
        bias2_arr = np.broadcast_to(bias2_row[None, :], (2, L * CHUNK)).copy()
        # rows are b-major stage order: row = b*7 + s, s = RT[t]
        sca_row = np.zeros(2 * LT, np.float32)
        scb_row = np.zeros(2 * LT, np.float32)
        for l in range(L):
            for t in range(T):
                for b in range(2):
                    r = l * RPL + b * T + RT[t]
                    sca_row[r] = scales[lt0 + l * T + t, 0]
                    scb_row[r] = scales[lt0 + l * T + t, 1]
        sca_arr = np.broadcast_to(sca_row[None, :], (128, 2 * LT)).copy()
        scb_arr = np.broadcast_to(scb_row[None, :], (128, 2 * LT)).copy()
        sba2_arr = np.zeros((16, L * 1024), np.float32)
        sbb2_arr = np.zeros((16, L * 1024), np.float32)
        for l in range(L):
            for t in range(T):
                for b in range(2):
                    r = b * T + RT[t]
                    sba2_arr[r, l * 1024 : (l + 1) * 1024] = (
                        scales[lt0 + l * T + t, 0] * bA2
                    )
                    sbb2_arr[r, l * 1024 : (l + 1) * 1024] = (
                        scales[lt0 + l * T + t, 1] * bB2
                    )
        in_maps.append(
            {
                "cond": cond_arr.astype(NPBF16),
                "wp": wp_c,
                "bias2": bias2_arr.astype(NPBF16),
                "wa1": wa1_arr.astype(NPBF16),
                "wb1": wb1_arr.astype(NPBF16),
                "wa2": wa2_arr.astype(NPBF16),
                "wb2": wb2_arr.astype(NPBF16),
                "ba1": ba1_arr,
                "bb1": bb1_arr,
                "sca": sca_arr.astype(NPBF16),
                "scb": scb_arr.astype(NPBF16),
                "sba2": sba2_arr.astype(NPBF16),
                "sbb2": sbb2_arr.astype(NPBF16),
                "ident": ident_arr.astype(NPBF16),
            }
        )
    return in_maps


def _ensure_ntff_hook():
    """Register the axon NTFF profile hook if the boot didn't (module was
    missing at boot time)."""
    import types

    ah = sys.modules.get("antenv.axon_hooks")
    if ah is None:
        ah = types.ModuleType("antenv.axon_hooks")
        ah._hook = None

        def _set(h, _m=ah):
            _m._hook = h

        def _get(_m=ah):
            return _m._hook

        ah.set_axon_ntff_profile_hook = _set
        ah.get_axon_ntff_profile_hook = _get
        sys.modules["antenv.axon_hooks"] = ah
        import antenv

        antenv.axon_hooks = ah
    if ah.get_axon_ntff_profile_hook() is None:
        if "/root/.axon_site" not in sys.path:
            sys.path.insert(0, "/root/.axon_site")
        from trn_agent_boot.trn_boot import _ntff_profile_via_ctypes

        hook = _ntff_profile_via_ctypes("/opt/axon/libaxon_pjrt.so")
        if hook is not None:
            ah.set_axon_ntff_profile_hook(hook)


def _run(inputs, trace=False):
    if trace:
        _ensure_ntff_hook()
    nc = _get_nc()
    in_maps = _marshal(inputs)
    res = run_bass_kernel_spmd(nc, in_maps, list(range(NCORES)), trace=trace)
    full = np.empty((2, NUM_LAYERS, LAYER_SIZE), np.float32)
    for c in range(NCORES):
        full[:, c * L : (c + 1) * L, :] = (
            res.results[c]["out"].astype(np.float32).reshape(2, L, LAYER_SIZE)
        )
    return full.reshape(2, -1), res


def kernel(**inputs) -> np.ndarray:
    out, _ = _run(inputs, trace=False)
    return out


# revision 34
# speedup vs baseline: 1.0872x; 1.0872x over previous
"""LoRA generator kernel for Trainium2, sharded over 8 NeuronCores by layer.

Reference computation:
  pe = (condition @ W_proj + b_proj)                        (B=2, 224, 512)
  A  = (gelu(pe@WA1+bA1) @ WA2 + bA2) -> (B, L, 7, 16, 64)
  Bm = (gelu(pe@WB1+bB1) @ WB2 + bB2) -> (B, L, 7, 64, 16)
  out per (b, layer): concat over t of [tile_cols(A)*scA (16 x in_d),
                                        tile_rows(B)*scB (out_d x 16)]

Each core handles 4 layers (28 of the 224 projections). All HBM traffic is
bf16 (W_proj cast on host, output upcast on host), halving the dominant DMA
bytes. Output pieces are written with 16KB descriptors: the decoder outputs
are scattered (one small SBUF->SBUF DMA per piece family) into wide tiles
whose partition rows hold the piece content base, then the periodic tiling
is materialized in-place by log2 doubling copies on the Vector/Scalar
engines, and each piece row streams out as contiguous 16KB reads.
"""
import sys

sys.path.insert(0, "/opt/trn_rl_repo")

import numpy as np
import ml_dtypes

import concourse.bass as bass
import concourse.bacc as bacc
import concourse.mybir as mybir
import concourse.tile as tile
from concourse.bass_utils import run_bass_kernel_spmd

F32 = mybir.dt.float32
BF16 = mybir.dt.bfloat16
NPBF16 = ml_dtypes.bfloat16

NCORES = 8
NUM_LAYERS = 32
RANK = 16
PED = 512
EMB = 384
T = 7
L = NUM_LAYERS // NCORES          # 4 layers per core
LT = L * T                        # 28 projections per core
RPL = 2 * T                       # 14 rows per layer; row = t*2 + b
CHUNK = T * PED                   # 3584 W_proj cols per layer

IN_DS = [4096, 4096, 4096, 4096, 4096, 4096, 11008]
OUT_DS = [4096, 1024, 1024, 4096, 11008, 11008, 4096]
A_SIZES = [16 * d for d in IN_DS]
B_SIZES = [16 * d for d in OUT_DS]
LAYER_SIZE = sum(A_SIZES) + sum(B_SIZES)   # 1150976
OFF_A = []
OFF_B = []
_o = 0
for _t in range(T):
    OFF_A.append(_o)
    _o += A_SIZES[_t]
    OFF_B.append(_o)
    _o += B_SIZES[_t]
OUT_SZ = 2 * L * LAYER_SIZE
LLS = L * LAYER_SIZE               # out stride between b=0 and b=1
WV = 8192                          # A/B tile row width (16KB bf16)

# CoreSim has no Gelu; debug harnesses set this to validate dataflow
SIM_GELU_IDENTITY = False

# Per-layer projections are processed in "stage order" s = rt(t) so that the
# t=6 (down_proj) rows land on partitions 0 and 7: its rank-scatter AP then
# has a stride-7 partition dim with zero offset, which the dependency
# tracker handles (k*pitch stride + nonzero partition offset does not).
RT = [1, 2, 3, 4, 5, 6, 0]         # stage slot of projection t
TOF = [6, 0, 1, 2, 3, 4, 5]        # projection t in stage slot s


def _build_nc():
    nc = bacc.Bacc(None, target_bir_lowering=False, debug=False)

    cond = nc.declare_dram_parameter("cond", [128, 6], BF16, isOutput=False)
    wp = nc.declare_dram_parameter("wp", [EMB, LT * PED], BF16, isOutput=False)
    bias2 = nc.declare_dram_parameter("bias2", [2, L * CHUNK], BF16, isOutput=False)
    wa1 = nc.declare_dram_parameter("wa1", [128, 1024], BF16, isOutput=False)
    wb1 = nc.declare_dram_parameter("wb1", [128, 1024], BF16, isOutput=False)
    wa2 = nc.declare_dram_parameter("wa2", [128, 2048], BF16, isOutput=False)
    wb2 = nc.declare_dram_parameter("wb2", [128, 2048], BF16, isOutput=False)
    ba1 = nc.declare_dram_parameter("ba1", [128, 2], F32, isOutput=False)
    bb1 = nc.declare_dram_parameter("bb1", [128, 2], F32, isOutput=False)
    sca = nc.declare_dram_parameter("sca", [128, 2 * LT], BF16, isOutput=False)
    scb = nc.declare_dram_parameter("scb", [128, 2 * LT], BF16, isOutput=False)
    sba2 = nc.declare_dram_parameter("sba2", [16, L * 1024], BF16, isOutput=False)
    sbb2 = nc.declare_dram_parameter("sbb2", [16, L * 1024], BF16, isOutput=False)
    ident = nc.declare_dram_parameter("ident", [16, 16], BF16, isOutput=False)
    out = nc.declare_dram_parameter("out", [OUT_SZ], BF16, isOutput=True)

    with tile.TileContext(nc) as tc:
        with (
            tc.tile_pool(name="const", bufs=1) as cpool,
            tc.tile_pool(name="wp", bufs=2) as wpool,
            tc.tile_pool(name="pe", bufs=2) as pepool,
            tc.tile_pool(name="dec", bufs=2) as decpool,
            tc.tile_pool(name="abuf", bufs=3) as apool,
            tc.tile_pool(name="bbuf", bufs=3) as bpool,
            tc.tile_pool(name="ps", bufs=1, space="PSUM") as ps,
        ):
            cond_sb = cpool.tile([128, 6], BF16)
            nc.sync.dma_start(cond_sb[:], cond[:])

            wa1_sb = cpool.tile([128, 1024], BF16)
            nc.sync.dma_start(wa1_sb[:], wa1[:])
            wb1_sb = cpool.tile([128, 1024], BF16)
            nc.scalar.dma_start(wb1_sb[:], wb1[:])
            wa2_sb = cpool.tile([128, 2048], BF16)
            nc.sync.dma_start(wa2_sb[:], wa2[:])
            wb2_sb = cpool.tile([128, 2048], BF16)
            nc.scalar.dma_start(wb2_sb[:], wb2[:])
            ba1_sb = cpool.tile([128, 2], F32)
            nc.sync.dma_start(ba1_sb[:], ba1[:])
            bb1_sb = cpool.tile([128, 2], F32)
            nc.scalar.dma_start(bb1_sb[:], bb1[:])
            sca_sb = cpool.tile([128, 2 * LT], BF16)
            nc.sync.dma_start(sca_sb[:], sca[:])
            scb_sb = cpool.tile([128, 2 * LT], BF16)
            nc.scalar.dma_start(scb_sb[:], scb[:])
            sba2_sb = cpool.tile([16, L * 1024], BF16)
            nc.sync.dma_start(sba2_sb[:], sba2[:])
            sbb2_sb = cpool.tile([16, L * 1024], BF16)
            nc.scalar.dma_start(sbb2_sb[:], sbb2[:])
            ident_sb = cpool.tile([16, 16], BF16)
            nc.sync.dma_start(ident_sb[:], ident[:])

            def load_chunk(l):
                tiles = []
                for kc in range(3):
                    t_ = wpool.tile([128, CHUNK], BF16, tag=f"wp{kc}", name=f"wp{kc}")
                    nc.gpsimd.dma_start(
                        t_[:],
                        wp[kc * 128 : (kc + 1) * 128, l * CHUNK : (l + 1) * CHUNK],
                    )
                    tiles.append(t_)
                return tiles

            def pe_layer(l, wp_t):
                """pe rows for layer l -> peT [128 ped-chunk x4, 14 rows] bf16."""
                pest = pepool.tile([2, CHUNK], BF16, tag="pest", name="pest")
                bias2L = pepool.tile([2, CHUNK], BF16, tag="b2", name="b2")
                nc.gpsimd.dma_start(
                    bias2L[:], bias2[0:2, l * CHUNK : (l + 1) * CHUNK]
                )
                for ltl in range(T):
                    p2 = ps.tile([2, PED], F32, tag=f"p2{ltl % 2}", name="p2")
                    for kc in range(3):
                        nc.tensor.matmul(
                            p2[:],
                            cond_sb[:, kc * 2 : kc * 2 + 2],
                            wp_t[kc][:, ltl * PED : (ltl + 1) * PED],
                            start=(kc == 0),
                            stop=(kc == 2),
                        )
                    nc.vector.tensor_add(
                        pest[0:2, ltl * PED : (ltl + 1) * PED],
                        p2[:],
                        bias2L[0:2, ltl * PED : (ltl + 1) * PED],
                    )
                # gather rows (b*7+t) onto partitions 0..13
                pe2 = pepool.tile([RPL, PED], BF16, tag="pe2", name="pe2")
                p16 = pe2[:, :].ap[0][0]
                pst = pest[:, :].ap[0][0]
                nc.gpsimd.dma_start(
                    bass.AP(pe2[:, :].tensor, 0, [[p16, RPL], [1, PED]]),
                    bass.AP(pest[:, :].tensor, 0, [[pst, 2], [PED, T], [1, PED]]),
                )
                peT = pepool.tile([128, 4 * RPL], BF16, tag="peT", name="peT")
                for mc in range(4):
                    tr = ps.tile([128, RPL], BF16, tag=f"tr{mc % 2}", name="tr")
                    nc.tensor.transpose(
                        tr[:],
                        pe2[:, mc * 128 : (mc + 1) * 128],
                        ident_sb[0:RPL, 0:RPL],
                    )
                    nc.vector.tensor_copy(peT[:, mc * RPL : (mc + 1) * RPL], tr[:])
                return peT

            def decode(l, peT, dec):
                """Decoder MLP for layer l -> o [14 rows, 1024] bf16 (scaled+bias)."""
                w1_sb, b1_sb, sc_sb, w2_sb, sb2_sb = (
                    (wa1_sb, ba1_sb, sca_sb, wa2_sb, sba2_sb)
                    if dec == 0
                    else (wb1_sb, bb1_sb, scb_sb, wb2_sb, sbb2_sb)
                )
                h_sb = []
                for mc in range(2):
                    hp = ps.tile([128, RPL], F32, tag=f"h{mc}", name="hp")
                    for kc in range(4):
                        nc.tensor.matmul(
                            hp[:],
                            w1_sb[:, kc * 256 + mc * 128 : kc * 256 + (mc + 1) * 128],
                            peT[:, kc * RPL : (kc + 1) * RPL],
                            start=(kc == 0),
                            stop=(kc == 3),
                        )
                    hs = decpool.tile(
                        [128, RPL], BF16, tag=f"hs{dec}{mc}", name="hs"
                    )
                    act = (
                        mybir.ActivationFunctionType.Identity
                        if SIM_GELU_IDENTITY
                        else mybir.ActivationFunctionType.Gelu
                    )
                    nc.scalar.activation(
                        hs[:], hp[:], act, bias=b1_sb[:, mc : mc + 1]
                    )
                    nc.vector.tensor_mul(
                        hs[:], hs[:], sc_sb[:, l * RPL : (l + 1) * RPL]
                    )
                    h_sb.append(hs)
                o_sb = decpool.tile([RPL, 1024], BF16, tag=f"o{dec}", name="o")
                for nh in range(2):
                    op = ps.tile([RPL, 512], F32, tag=f"o{nh}", name="op")
                    for kc in range(2):
                        nc.tensor.matmul(
                            op[:],
                            h_sb[kc][:],
                            w2_sb[:, kc * 1024 + nh * 512 : kc * 1024 + (nh + 1) * 512],
                            start=(kc == 0),
                            stop=(kc == 1),
                        )
                    nc.vector.tensor_add(
                        o_sb[:, nh * 512 : (nh + 1) * 512],
                        op[:],
                        sb2_sb[0:RPL, l * 1024 + nh * 512 : l * 1024 + (nh + 1) * 512],
                    )
                return o_sb

            def emit_layer(l, peT):
                oa = decode(l, peT, 0)
                ob = decode(l, peT, 1)

                atile = apool.tile([128, WV], BF16, tag="atile", name="atile")
                btile = bpool.tile([112, WV], BF16, tag="btile", name="btile")
                pa = oa[:, :].ap[0][0]
                pb = ob[:, :].ap[0][0]
                pax = atile[:, :].ap[0][0]
                pbx = btile[:, :].ap[0][0]
                oa_t = oa[:, :].tensor
                ob_t = ob[:, :].tensor
                atile_t = atile[:, :].tensor
                btile_t = btile[:, :].tensor

                # --- seed scatters (SBUF->SBUF) ---
                # rows are b-major stage order: row = b*7 + RT[t]
                # A t<6 (rows 1..6): partition 48b+8t+rp gets rank 2rp at col 0,
                # rank 2rp+1 at col 4096
                for b in range(2):
                    for half in range(2):
                        nc.gpsimd.dma_start(
                            bass.AP(
                                atile_t,
                                48 * b * pax + half * 4096,
                                [[pax, 48], [1, 64]],
                            ),
                            bass.AP(
                                oa_t,
                                (7 * b + 1) * pa + half * 64,
                                [[pa, 6], [128, 8], [1, 64]],
                            ),
                        )
                # A t=6 (rows 0, 7): partition 96+16b+r gets rank r at col 0;
                # the col-4096 seed copy is done by the vector engine
                nc.gpsimd.dma_start(
                    bass.AP(atile_t, 96 * pax, [[pax, 32], [1, 64]]),
                    bass.AP(oa_t, 0, [[7 * pa, 2], [64, 16], [1, 64]]),
                )
                # B: partition 56b+8t+copy gets the 1024-elem base block
                nc.gpsimd.dma_start(
                    bass.AP(btile_t, 0, [[pbx, 112], [1, 1024]]),
                    bass.AP(ob_t, 0, [[pb, 14], [0, 8], [1, 1024]]),
                )

                # --- widen by doubling (A on vector, B on scalar) ---
                nc.vector.tensor_copy(
                    atile[96:128, 4096:4160], atile[96:128, 0:64]
                )
                w = 64
                while w < 4096:
                    nc.vector.tensor_copy(atile[:, w : 2 * w], atile[:, 0:w])
                    nc.vector.tensor_copy(
                        atile[:, 4096 + w : 4096 + 2 * w], atile[:, 4096 : 4096 + w]
                    )
                    w *= 2
                w = 1024
                while w < WV:
                    nc.scalar.copy(btile[:, w : 2 * w], btile[:, 0:w])
                    w *= 2

                # --- output writes ---
                # All B copies of a piece are identical, so any rep may read
                # any copy: merges pick copy windows that make the source a
                # single contiguous partition run.
                base = l * LAYER_SIZE
                stA45 = A_SIZES[4] + B_SIZES[4]   # A t4 -> t5 stride (241664)
                stB45 = B_SIZES[4] + A_SIZES[5]   # B t4 -> t5 stride (241664)
                for b in range(2):
                    bo = base + b * LLS
                    pA = 48 * b * pax
                    pB = 56 * b * pbx
                    # A t0: 8 x 16KB
                    nc.sync.dma_start(
                        bass.AP(out, bo + OFF_A[0], [[8192, 8], [1, 8192]]),
                        bass.AP(atile_t, pA, [[pax, 8], [1, 8192]]),
                    )
                    # A t1..t3 (uniform stride 81920): 24 x 16KB
                    nc.sync.dma_start(
                        bass.AP(
                            out, bo + OFF_A[1], [[81920, 3], [8192, 8], [1, 8192]]
                        ),
                        bass.AP(atile_t, pA + 8 * pax, [[pax, 24], [1, 8192]]),
                    )
                    # A t4, t5: 16 x 16KB
                    nc.sync.dma_start(
                        bass.AP(
                            out, bo + OFF_A[4], [[stA45, 2], [8192, 8], [1, 8192]]
                        ),
                        bass.AP(atile_t, pA + 32 * pax, [[pax, 16], [1, 8192]]),
                    )
                    # A t6: 11008-elem rows as two 5504 chunks (period 64
                    # divides 5504; the widened row is periodic through 8192)
                    nc.sync.dma_start(
                        bass.AP(
                            out, bo + OFF_A[6], [[11008, 16], [5504, 2], [1, 5504]]
                        ),
                        bass.AP(
                            atile_t,
                            (96 + 16 * b) * pax,
                            [[pax, 16], [0, 2], [1, 5504]],
                        ),
                    )
                    # B piece for projection t sits at partitions 56b + 8*RT[t]
                    # B t0 (slot 1): 8 x 16KB
                    nc.scalar.dma_start(
                        bass.AP(out, bo + OFF_B[0], [[8192, 8], [1, 8192]]),
                        bass.AP(
                            btile_t, pB + 8 * RT[0] * pbx, [[pbx, 8], [1, 8192]]
                        ),
                    )
                    # B t1+t2 merged (slots 2,3): reps read t1 copies 6,7 then
                    # t2 copies 0,1 = contiguous partitions 22..25
                    nc.scalar.dma_start(
                        bass.AP(
                            out, bo + OFF_B[1], [[81920, 2], [8192, 2], [1, 8192]]
                        ),
                        bass.AP(
                            btile_t,
                            pB + (8 * RT[1] + 6) * pbx,
                            [[pbx, 4], [1, 8192]],
                        ),
                    )
                    # B t3 (slot 4): 8 x 16KB
                    nc.scalar.dma_start(
                        bass.AP(out, bo + OFF_B[3], [[8192, 8], [1, 8192]]),
                        bass.AP(
                            btile_t, pB + 8 * RT[3] * pbx, [[pbx, 8], [1, 8192]]
                        ),
                    )
                    # B t4+t5 (slots 5,6): 21.5 reps each = 16 + 4 + 1 + half.
                    # pass1 per t: each of the 8 copies read twice = reps 0..15
                    for t in (4, 5):
                        nc.sync.dma_start(
                            bass.AP(out, bo + OFF_B[t], [[8192, 16], [1, 8192]]),
                            bass.AP(
                                btile_t,
                                pB + 8 * RT[t] * pbx,
                                [[pbx, 8], [0, 2], [1, 8192]],
                            ),
                        )
                    # pass2: reps 16..19 <- t4 copies 4..7, t5 copies 0..3
                    nc.scalar.dma_start(
                        bass.AP(
                            out,
                            bo + OFF_B[4] + 131072,
                            [[stB45, 2], [8192, 4], [1, 8192]],
                        ),
                        bass.AP(
                            btile_t,
                            pB + (8 * RT[4] + 4) * pbx,
                            [[pbx, 8], [1, 8192]],
                        ),
                    )
                    # pass3: rep 20 + half rep <- t4 copy 7, t5 copy 0
                    nc.gpsimd.dma_start(
                        bass.AP(out, bo + OFF_B[4] + 163840, [[stB45, 2], [1, 8192]]),
                        bass.AP(
                            btile_t,
                            pB + (8 * RT[4] + 7) * pbx,
                            [[pbx, 2], [1, 8192]],
                        ),
                    )
                    nc.gpsimd.dma_start(
                        bass.AP(out, bo + OFF_B[4] + 172032, [[stB45, 2], [1, 4096]]),
                        bass.AP(
                            btile_t,
                            pB + (8 * RT[4] + 7) * pbx,
                            [[pbx, 2], [1, 4096]],
                        ),
                    )
                    # B t6 (slot 0): 8 x 16KB
                    nc.scalar.dma_start(
                        bass.AP(out, bo + OFF_B[6], [[8192, 8], [1, 8192]]),
                        bass.AP(btile_t, pB, [[pbx, 8], [1, 8192]]),
                    )

            # ---- main pipeline ----
            chunks = [load_chunk(0), load_chunk(1)]
            for l in range(L):
                peT = pe_layer(l, chunks[l])
                if l + 2 < L:
                    chunks.append(load_chunk(l + 2))
                emit_layer(l, peT)

    nc.finalize()
    return nc


_NC = None


def _get_nc():
    global _NC
    if _NC is None:
        _NC = _build_nc()
    return _NC


def _marshal(inputs):
    """Build the per-core input maps from full inputs."""
    condition = np.asarray(inputs["condition"], np.float32)
    W_proj = np.asarray(inputs["W_proj"], np.float32)
    b_proj = np.asarray(inputs["b_proj"], np.float32)
    WA1 = np.asarray(inputs["WA1"], np.float32)
    bA1 = np.asarray(inputs["bA1"], np.float32)
    WA2 = np.asarray(inputs["WA2"], np.float32)
    bA2 = np.asarray(inputs["bA2"], np.float32)
    WB1 = np.asarray(inputs["WB1"], np.float32)
    bB1 = np.asarray(inputs["bB1"], np.float32)
    WB2 = np.asarray(inputs["WB2"], np.float32)
    bB2 = np.asarray(inputs["bB2"], np.float32)
    scales = np.asarray(inputs["scales"], np.float32)

    cond_arr = np.zeros((128, 6), np.float32)
    for kc in range(3):
        cond_arr[:, kc * 2 : kc * 2 + 2] = condition[:, kc * 128 : (kc + 1) * 128].T
    wa1_arr = np.zeros((128, 1024), np.float32)
    wb1_arr = np.zeros((128, 1024), np.float32)
    for kc in range(4):
        wa1_arr[:, kc * 256 : (kc + 1) * 256] = WA1[kc * 128 : (kc + 1) * 128, :]
        wb1_arr[:, kc * 256 : (kc + 1) * 256] = WB1[kc * 128 : (kc + 1) * 128, :]
    wa2_arr = np.zeros((128, 2048), np.float32)
    wb2_arr = np.zeros((128, 2048), np.float32)
    for kc in range(2):
        wa2_arr[:, kc * 1024 : (kc + 1) * 1024] = WA2[kc * 128 : (kc + 1) * 128, :]
        wb2_arr[:, kc * 1024 : (kc + 1) * 1024] = WB2[kc * 128 : (kc + 1) * 128, :]
    ba1_arr = np.ascontiguousarray(bA1.reshape(2, 128).T)
    bb1_arr = np.ascontiguousarray(bB1.reshape(2, 128).T)
    ident_arr = np.eye(16, dtype=np.float32)

    in_maps = []
    for c in range(NCORES):
        lt0 = c * LT
        # reorder each layer's 7 projections into stage order s (t = TOF[s])
        wp_full = W_proj[:, lt0 * PED : (lt0 + LT) * PED].reshape(EMB, L, T, PED)
        bp_full = b_proj[lt0 * PED : (lt0 + LT) * PED].reshape(L, T, PED)
        wp_c = np.ascontiguousarray(
            wp_full[:, :, TOF, :].reshape(EMB, LT * PED).astype(NPBF16)
        )
        bias2_row = np.ascontiguousarray(bp_full[:, TOF, :].reshape(L * CHUNK))
        bias2_arr = np.broadcast_to(bias2_row[None, :], (2, L * CHUNK)).copy()
        # rows are b-major stage order: row = b*7 + s, s = RT[t]
        sca_row = np.zeros(2 * LT, np.float32)
        scb_row = np.zeros(2 * LT, np.float32)
        for l in range(L):
            for t in range(T):
                for b in range(2):
                    r = l * RPL + b * T + RT[t]
                    sca_row[r] = scales[lt0 + l * T + t, 0]
                    scb_row[r] = scales[lt0 + l * T + t, 1]
        sca_arr = np.broadcast_to(sca_row[None, :], (128, 2 * LT)).copy()
        scb_arr = np.broadcast_to(scb_row[None, :], (128, 2 * LT)).copy()
        sba2_arr = np.zeros((16, L * 1024), np.float32)
        sbb2_arr = np.zeros((16, L * 1024), np.float32)
        for l in range(L):
            for t in range(T):
                for b in range(2):
                    r = b * T + RT[t]
                    sba2_arr[r, l * 1024 : (l + 1) * 1024] = (
                        scales[lt0 + l * T + t, 0] * bA2
                    )
                    sbb2_arr[r, l * 1024 : (l + 1) * 1024] = (
                        scales[lt0 + l * T + t, 1] * bB2
                    )
        in_maps.append(
            {
                "cond": cond_arr.astype(NPBF16),
                "wp": wp_c,
                "bias2": bias2_arr.astype(NPBF16),
                "wa1": wa1_arr.astype(NPBF16),
                "wb1": wb1_arr.astype(NPBF16),
                "wa2": wa2_arr.astype(NPBF16),
                "wb2": wb2_arr.astype(NPBF16),
                "ba1": ba1_arr,
                "bb1": bb1_arr,
                "sca": sca_arr.astype(NPBF16),
                "scb": scb_arr.astype(NPBF16),
                "sba2": sba2_arr.astype(NPBF16),
                "sbb2": sbb2_arr.astype(NPBF16),
                "ident": ident_arr.astype(NPBF16),
            }
        )
    return in_maps


def _ensure_ntff_hook():
    """Register the axon NTFF profile hook if the boot didn't (module was
    missing at boot time)."""
    import types

    ah = sys.modules.get("antenv.axon_hooks")
    if ah is None:
        ah = types.ModuleType("antenv.axon_hooks")
        ah._hook = None

        def _set(h, _m=ah):
            _m._hook = h

        def _get(_m=ah):
            return _m._hook

        ah.set_axon_ntff_profile_hook = _set
        ah.get_axon_ntff_profile_hook = _get
        sys.modules["antenv.axon_hooks"] = ah
        import antenv

        antenv.axon_hooks = ah
    if ah.get_axon_ntff_profile_hook() is None:
        if "/root/.axon_site" not in sys.path:
            sys.path.insert(0, "/root/.axon_site")
        from trn_agent_boot.trn_boot import _ntff_profile_via_ctypes

        hook = _ntff_profile_via_ctypes("/opt/axon/libaxon_pjrt.so")
        if hook is not None:
            ah.set_axon_ntff_profile_hook(hook)


def _run(inputs, trace=False):
    if trace:
        _ensure_ntff_hook()
    nc = _get_nc()
    in_maps = _marshal(inputs)
    res = run_bass_kernel_spmd(nc, in_maps, list(range(NCORES)), trace=trace)
    full = np.empty((2, NUM_LAYERS, LAYER_SIZE), np.float32)
    for c in range(NCORES):
        full[:, c * L : (c + 1) * L, :] = (
            res.results[c]["out"].astype(np.float32).reshape(2, L, LAYER_SIZE)
        )
    return full.reshape(2, -1), res


def kernel(**inputs) -> np.ndarray:
    out, _ = _run(inputs, trace=False)
    return out


# revision 37
# speedup vs baseline: 1.3103x; 1.2052x over previous
"""LoRA generator kernel for Trainium2, sharded over 8 NeuronCores by layer.

Reference computation:
  pe = (condition @ W_proj + b_proj)                        (B=2, 224, 512)
  A  = (gelu(pe@WA1+bA1) @ WA2 + bA2) -> (B, L, 7, 16, 64)
  Bm = (gelu(pe@WB1+bB1) @ WB2 + bB2) -> (B, L, 7, 64, 16)
  out per (b, layer): concat over t of [tile_cols(A)*scA (16 x in_d),
                                        tile_rows(B)*scB (out_d x 16)]

Each core handles 4 layers (28 of the 224 projections). All HBM traffic is
bf16 (W_proj cast on host, output upcast on host), halving the dominant DMA
bytes. Output pieces are written with 16KB descriptors: the decoder outputs
are scattered (one small SBUF->SBUF DMA per piece family) into wide tiles
whose partition rows hold the piece content base, then the periodic tiling
is materialized in-place by log2 doubling copies on the Vector/Scalar
engines, and each piece row streams out as contiguous 16KB reads.
"""
import sys

sys.path.insert(0, "/opt/trn_rl_repo")

import numpy as np
import ml_dtypes

import concourse.bass as bass
import concourse.bacc as bacc
import concourse.mybir as mybir
import concourse.tile as tile
from concourse.bass_utils import run_bass_kernel_spmd

F32 = mybir.dt.float32
BF16 = mybir.dt.bfloat16
NPBF16 = ml_dtypes.bfloat16

NCORES = 8
NUM_LAYERS = 32
RANK = 16
PED = 512
EMB = 384
T = 7
L = NUM_LAYERS // NCORES          # 4 layers per core
LT = L * T                        # 28 projections per core
RPL = 2 * T                       # 14 rows per layer; row = t*2 + b
CHUNK = T * PED                   # 3584 W_proj cols per layer

IN_DS = [4096, 4096, 4096, 4096, 4096, 4096, 11008]
OUT_DS = [4096, 1024, 1024, 4096, 11008, 11008, 4096]
A_SIZES = [16 * d for d in IN_DS]
B_SIZES = [16 * d for d in OUT_DS]
LAYER_SIZE = sum(A_SIZES) + sum(B_SIZES)   # 1150976
OFF_A = []
OFF_B = []
_o = 0
for _t in range(T):
    OFF_A.append(_o)
    _o += A_SIZES[_t]
    OFF_B.append(_o)
    _o += B_SIZES[_t]
OUT_SZ = 2 * L * LAYER_SIZE
LLS = L * LAYER_SIZE               # out stride between b=0 and b=1
WV = 8192                          # A/B tile row width (16KB bf16)

# CoreSim has no Gelu; debug harnesses set this to validate dataflow
SIM_GELU_IDENTITY = False

# Per-layer projections are processed in "stage order" s = rt(t) so that the
# t=6 (down_proj) rows land on partitions 0 and 7: its rank-scatter AP then
# has a stride-7 partition dim with zero offset, which the dependency
# tracker handles (k*pitch stride + nonzero partition offset does not).
RT = [1, 2, 3, 4, 5, 6, 0]         # stage slot of projection t
TOF = [6, 0, 1, 2, 3, 4, 5]        # projection t in stage slot s


def _build_nc():
    nc = bacc.Bacc(None, target_bir_lowering=False, debug=False)

    cond = nc.declare_dram_parameter("cond", [128, 6], BF16, isOutput=False)
    wp = nc.declare_dram_parameter("wp", [EMB, LT * PED], BF16, isOutput=False)
    bias2 = nc.declare_dram_parameter("bias2", [2, L * CHUNK], BF16, isOutput=False)
    wa1 = nc.declare_dram_parameter("wa1", [128, 1024], BF16, isOutput=False)
    wb1 = nc.declare_dram_parameter("wb1", [128, 1024], BF16, isOutput=False)
    wa2 = nc.declare_dram_parameter("wa2", [128, 2048], BF16, isOutput=False)
    wb2 = nc.declare_dram_parameter("wb2", [128, 2048], BF16, isOutput=False)
    ba1 = nc.declare_dram_parameter("ba1", [128, 2], F32, isOutput=False)
    bb1 = nc.declare_dram_parameter("bb1", [128, 2], F32, isOutput=False)
    sca = nc.declare_dram_parameter("sca", [128, 2 * LT], BF16, isOutput=False)
    scb = nc.declare_dram_parameter("scb", [128, 2 * LT], BF16, isOutput=False)
    sba2 = nc.declare_dram_parameter("sba2", [16, L * 1024], BF16, isOutput=False)
    sbb2 = nc.declare_dram_parameter("sbb2", [16, L * 1024], BF16, isOutput=False)
    ident = nc.declare_dram_parameter("ident", [16, 16], BF16, isOutput=False)
    out = nc.declare_dram_parameter("out", [OUT_SZ], BF16, isOutput=True)

    with tile.TileContext(nc) as tc:
        with (
            tc.tile_pool(name="const", bufs=1) as cpool,
            tc.tile_pool(name="wp", bufs=2) as wpool,
            tc.tile_pool(name="pe", bufs=2) as pepool,
            tc.tile_pool(name="dec", bufs=2) as decpool,
            tc.tile_pool(name="abuf", bufs=3) as apool,
            tc.tile_pool(name="bbuf", bufs=3) as bpool,
            tc.tile_pool(name="ps", bufs=1, space="PSUM") as ps,
        ):
            cond_sb = cpool.tile([128, 6], BF16)
            nc.sync.dma_start(cond_sb[:], cond[:])

            wa1_sb = cpool.tile([128, 1024], BF16)
            nc.sync.dma_start(wa1_sb[:], wa1[:])
            wb1_sb = cpool.tile([128, 1024], BF16)
            nc.scalar.dma_start(wb1_sb[:], wb1[:])
            wa2_sb = cpool.tile([128, 2048], BF16)
            nc.sync.dma_start(wa2_sb[:], wa2[:])
            wb2_sb = cpool.tile([128, 2048], BF16)
            nc.scalar.dma_start(wb2_sb[:], wb2[:])
            ba1_sb = cpool.tile([128, 2], F32)
            nc.sync.dma_start(ba1_sb[:], ba1[:])
            bb1_sb = cpool.tile([128, 2], F32)
            nc.scalar.dma_start(bb1_sb[:], bb1[:])
            sca_sb = cpool.tile([128, 2 * LT], BF16)
            nc.sync.dma_start(sca_sb[:], sca[:])
            scb_sb = cpool.tile([128, 2 * LT], BF16)
            nc.scalar.dma_start(scb_sb[:], scb[:])
            sba2_sb = cpool.tile([16, L * 1024], BF16)
            nc.sync.dma_start(sba2_sb[:], sba2[:])
            sbb2_sb = cpool.tile([16, L * 1024], BF16)
            nc.scalar.dma_start(sbb2_sb[:], sbb2[:])
            ident_sb = cpool.tile([16, 16], BF16)
            nc.sync.dma_start(ident_sb[:], ident[:])

            def load_chunk(l):
                tiles = []
                for kc in range(3):
                    t_ = wpool.tile([128, CHUNK], BF16, tag=f"wp{kc}", name=f"wp{kc}")
                    nc.gpsimd.dma_start(
                        t_[:],
                        wp[kc * 128 : (kc + 1) * 128, l * CHUNK : (l + 1) * CHUNK],
                    )
                    tiles.append(t_)
                return tiles

            def pe_layer(l, wp_t):
                """pe rows for layer l -> peT [128 ped-chunk x4, 14 rows] bf16."""
                pest = pepool.tile([2, CHUNK], BF16, tag="pest", name="pest")
                bias2L = pepool.tile([2, CHUNK], BF16, tag="b2", name="b2")
                nc.scalar.dma_start(
                    bias2L[:], bias2[0:2, l * CHUNK : (l + 1) * CHUNK]
                )
                for ltl in range(T):
                    p2 = ps.tile([2, PED], F32, tag=f"p2{ltl % 2}", name="p2")
                    for kc in range(3):
                        nc.tensor.matmul(
                            p2[:],
                            cond_sb[:, kc * 2 : kc * 2 + 2],
                            wp_t[kc][:, ltl * PED : (ltl + 1) * PED],
                            start=(kc == 0),
                            stop=(kc == 2),
                        )
                    nc.vector.tensor_add(
                        pest[0:2, ltl * PED : (ltl + 1) * PED],
                        p2[:],
                        bias2L[0:2, ltl * PED : (ltl + 1) * PED],
                    )
                # gather rows (b*7+t) onto partitions 0..13
                pe2 = pepool.tile([RPL, PED], BF16, tag="pe2", name="pe2")
                p16 = pe2[:, :].ap[0][0]
                pst = pest[:, :].ap[0][0]
                nc.sync.dma_start(
                    bass.AP(pe2[:, :].tensor, 0, [[p16, RPL], [1, PED]]),
                    bass.AP(pest[:, :].tensor, 0, [[pst, 2], [PED, T], [1, PED]]),
                )
                peT = pepool.tile([128, 4 * RPL], BF16, tag="peT", name="peT")
                for mc in range(4):
                    tr = ps.tile([128, RPL], BF16, tag=f"tr{mc % 2}", name="tr")
                    nc.tensor.transpose(
                        tr[:],
                        pe2[:, mc * 128 : (mc + 1) * 128],
                        ident_sb[0:RPL, 0:RPL],
                    )
                    nc.vector.tensor_copy(peT[:, mc * RPL : (mc + 1) * RPL], tr[:])
                return peT

            def decode(l, peT, dec):
                """Decoder MLP for layer l -> o [14 rows, 1024] bf16 (scaled+bias)."""
                w1_sb, b1_sb, sc_sb, w2_sb, sb2_sb = (
                    (wa1_sb, ba1_sb, sca_sb, wa2_sb, sba2_sb)
                    if dec == 0
                    else (wb1_sb, bb1_sb, scb_sb, wb2_sb, sbb2_sb)
                )
                h_sb = []
                for mc in range(2):
                    hp = ps.tile([128, RPL], F32, tag=f"h{mc}", name="hp")
                    for kc in range(4):
                        nc.tensor.matmul(
                            hp[:],
                            w1_sb[:, kc * 256 + mc * 128 : kc * 256 + (mc + 1) * 128],
                            peT[:, kc * RPL : (kc + 1) * RPL],
                            start=(kc == 0),
                            stop=(kc == 3),
                        )
                    hs = decpool.tile(
                        [128, RPL], BF16, tag=f"hs{dec}{mc}", name="hs"
                    )
                    act = (
                        mybir.ActivationFunctionType.Identity
                        if SIM_GELU_IDENTITY
                        else mybir.ActivationFunctionType.Gelu
                    )
                    nc.scalar.activation(
                        hs[:], hp[:], act, bias=b1_sb[:, mc : mc + 1]
                    )
                    nc.vector.tensor_mul(
                        hs[:], hs[:], sc_sb[:, l * RPL : (l + 1) * RPL]
                    )
                    h_sb.append(hs)
                o_sb = decpool.tile([RPL, 1024], BF16, tag=f"o{dec}", name="o")
                for nh in range(2):
                    op = ps.tile([RPL, 512], F32, tag=f"o{nh}", name="op")
                    for kc in range(2):
                        nc.tensor.matmul(
                            op[:],
                            h_sb[kc][:],
                            w2_sb[:, kc * 1024 + nh * 512 : kc * 1024 + (nh + 1) * 512],
                            start=(kc == 0),
                            stop=(kc == 1),
                        )
                    nc.vector.tensor_add(
                        o_sb[:, nh * 512 : (nh + 1) * 512],
                        op[:],
                        sb2_sb[0:RPL, l * 1024 + nh * 512 : l * 1024 + (nh + 1) * 512],
                    )
                return o_sb

            def emit_layer(l, peT):
                oa = decode(l, peT, 0)
                ob = decode(l, peT, 1)

                atile = apool.tile([128, WV], BF16, tag="atile", name="atile")
                btile = bpool.tile([112, WV], BF16, tag="btile", name="btile")
                pa = oa[:, :].ap[0][0]
                pb = ob[:, :].ap[0][0]
                pax = atile[:, :].ap[0][0]
                pbx = btile[:, :].ap[0][0]
                oa_t = oa[:, :].tensor
                ob_t = ob[:, :].tensor
                atile_t = atile[:, :].tensor
                btile_t = btile[:, :].tensor

                # --- seed scatters (SBUF->SBUF) ---
                # rows are b-major stage order: row = b*7 + RT[t]
                # A t<6 (rows 1..6): partition 48b+8t+rp gets rank 2rp at col 0,
                # rank 2rp+1 at col 4096
                for b in range(2):
                    hw = nc.sync if b == 0 else nc.scalar
                    for half in range(2):
                        hw.dma_start(
                            bass.AP(
                                atile_t,
                                48 * b * pax + half * 4096,
                                [[pax, 48], [1, 64]],
                            ),
                            bass.AP(
                                oa_t,
                                (7 * b + 1) * pa + half * 64,
                                [[pa, 6], [128, 8], [1, 64]],
                            ),
                        )
                # A t=6 (rows 0, 7): partition 96+16b+r gets rank r at col 0;
                # the col-4096 seed copy is done by the vector engine
                nc.sync.dma_start(
                    bass.AP(atile_t, 96 * pax, [[pax, 32], [1, 64]]),
                    bass.AP(oa_t, 0, [[7 * pa, 2], [64, 16], [1, 64]]),
                )
                # B: partition 56b+8t+copy gets the 1024-elem base block
                nc.scalar.dma_start(
                    bass.AP(btile_t, 0, [[pbx, 112], [1, 1024]]),
                    bass.AP(ob_t, 0, [[pb, 14], [0, 8], [1, 1024]]),
                )

                # --- widen by doubling (A on vector, B on scalar) ---
                nc.vector.tensor_copy(
                    atile[96:128, 4096:4160], atile[96:128, 0:64]
                )
                w = 64
                while w < 4096:
                    nc.vector.tensor_copy(atile[:, w : 2 * w], atile[:, 0:w])
                    nc.vector.tensor_copy(
                        atile[:, 4096 + w : 4096 + 2 * w], atile[:, 4096 : 4096 + w]
                    )
                    w *= 2
                w = 1024
                while w < WV:
                    nc.scalar.copy(btile[:, w : 2 * w], btile[:, 0:w])
                    w *= 2

                # --- output writes ---
                # All B copies of a piece are identical, so any rep may read
                # any copy: merges pick copy windows that make the source a
                # single contiguous partition run. Bulk streams go b0->sync,
                # b1->scalar, small/odd pieces -> gpsimd (SWDGE).
                base = l * LAYER_SIZE
                stA45 = A_SIZES[4] + B_SIZES[4]   # A t4 -> t5 stride (241664)
                stB45 = B_SIZES[4] + A_SIZES[5]   # B t4 -> t5 stride (241664)
                for b in range(2):
                    bo = base + b * LLS
                    pA = 48 * b * pax
                    pB = 56 * b * pbx
                    hw = nc.sync if b == 0 else nc.scalar
                    # A t0: 8 x 16KB (gpsimd)
                    nc.gpsimd.dma_start(
                        bass.AP(out, bo + OFF_A[0], [[8192, 8], [1, 8192]]),
                        bass.AP(atile_t, pA, [[pax, 8], [1, 8192]]),
                    )
                    # A t1..t3 (uniform stride 81920): 24 x 16KB
                    hw.dma_start(
                        bass.AP(
                            out, bo + OFF_A[1], [[81920, 3], [8192, 8], [1, 8192]]
                        ),
                        bass.AP(atile_t, pA + 8 * pax, [[pax, 24], [1, 8192]]),
                    )
                    # A t4, t5: 16 x 16KB
                    hw.dma_start(
                        bass.AP(
                            out, bo + OFF_A[4], [[stA45, 2], [8192, 8], [1, 8192]]
                        ),
                        bass.AP(atile_t, pA + 32 * pax, [[pax, 16], [1, 8192]]),
                    )
                    # A t6: 11008-elem rows as two 5504 chunks (period 64
                    # divides 5504; the widened row is periodic through 8192)
                    hw.dma_start(
                        bass.AP(
                            out, bo + OFF_A[6], [[11008, 16], [5504, 2], [1, 5504]]
                        ),
                        bass.AP(
                            atile_t,
                            (96 + 16 * b) * pax,
                            [[pax, 16], [0, 2], [1, 5504]],
                        ),
                    )
                    # B piece for projection t sits at partitions 56b + 8*RT[t]
                    # B t0 (slot 1): 8 x 16KB
                    hw.dma_start(
                        bass.AP(out, bo + OFF_B[0], [[8192, 8], [1, 8192]]),
                        bass.AP(
                            btile_t, pB + 8 * RT[0] * pbx, [[pbx, 8], [1, 8192]]
                        ),
                    )
                    # B t1+t2 merged (slots 2,3): reps read t1 copies 6,7 then
                    # t2 copies 0,1 = contiguous partitions (gpsimd)
                    nc.gpsimd.dma_start(
                        bass.AP(
                            out, bo + OFF_B[1], [[81920, 2], [8192, 2], [1, 8192]]
                        ),
                        bass.AP(
                            btile_t,
                            pB + (8 * RT[1] + 6) * pbx,
                            [[pbx, 4], [1, 8192]],
                        ),
                    )
                    # B t3 (slot 4): 8 x 16KB
                    hw.dma_start(
                        bass.AP(out, bo + OFF_B[3], [[8192, 8], [1, 8192]]),
                        bass.AP(
                            btile_t, pB + 8 * RT[3] * pbx, [[pbx, 8], [1, 8192]]
                        ),
                    )
                    # B t4+t5 (slots 5,6): 21.5 reps of 8192 = 16 + 4 + 1 + half.
                    # pass1 per t: each of the 8 copies read twice = reps 0..15
                    for t in (4, 5):
                        hw.dma_start(
                            bass.AP(out, bo + OFF_B[t], [[8192, 16], [1, 8192]]),
                            bass.AP(
                                btile_t,
                                pB + 8 * RT[t] * pbx,
                                [[pbx, 8], [0, 2], [1, 8192]],
                            ),
                        )
                    # pass2: reps 16..19 <- t4 copies 4..7, t5 copies 0..3
                    nc.gpsimd.dma_start(
                        bass.AP(
                            out,
                            bo + OFF_B[4] + 131072,
                            [[stB45, 2], [8192, 4], [1, 8192]],
                        ),
                        bass.AP(
                            btile_t,
                            pB + (8 * RT[4] + 4) * pbx,
                            [[pbx, 8], [1, 8192]],
                        ),
                    )
                    # pass3: rep 20 + half rep <- t4 copy 7, t5 copy 0
                    nc.gpsimd.dma_start(
                        bass.AP(out, bo + OFF_B[4] + 163840, [[stB45, 2], [1, 8192]]),
                        bass.AP(
                            btile_t,
                            pB + (8 * RT[4] + 7) * pbx,
                            [[pbx, 2], [1, 8192]],
                        ),
                    )
                    nc.gpsimd.dma_start(
                        bass.AP(out, bo + OFF_B[4] + 172032, [[stB45, 2], [1, 4096]]),
                        bass.AP(
                            btile_t,
                            pB + (8 * RT[4] + 7) * pbx,
                            [[pbx, 2], [1, 4096]],
                        ),
                    )
                    # B t6 (slot 0): 8 x 16KB (gpsimd)
                    nc.gpsimd.dma_start(
                        bass.AP(out, bo + OFF_B[6], [[8192, 8], [1, 8192]]),
                        bass.AP(btile_t, pB, [[pbx, 8], [1, 8192]]),
                    )

            # ---- main pipeline ----
            chunks = [load_chunk(0), load_chunk(1)]
            for l in range(L):
                peT = pe_layer(l, chunks[l])
                emit_layer(l, peT)
                if l + 2 < L:
                    chunks.append(load_chunk(l + 2))

    nc.finalize()
    return nc


_NC = None


def _get_nc():
    global _NC
    if _NC is None:
        _NC = _build_nc()
    return _NC


def _marshal(inputs):
    """Build the per-core input maps from full inputs."""
    condition = np.asarray(inputs["condition"], np.float32)
    W_proj = np.asarray(inputs["W_proj"], np.float32)
    b_proj = np.asarray(inputs["b_proj"], np.float32)
    WA1 = np.asarray(inputs["WA1"], np.float32)
    bA1 = np.asarray(inputs["bA1"], np.float32)
    WA2 = np.asarray(inputs["WA2"], np.float32)
    bA2 = np.asarray(inputs["bA2"], np.float32)
    WB1 = np.asarray(inputs["WB1"], np.float32)
    bB1 = np.asarray(inputs["bB1"], np.float32)
    WB2 = np.asarray(inputs["WB2"], np.float32)
    bB2 = np.asarray(inputs["bB2"], np.float32)
    scales = np.asarray(inputs["scales"], np.float32)

    cond_arr = np.zeros((128, 6), np.float32)
    for kc in range(3):
        cond_arr[:, kc * 2 : kc * 2 + 2] = condition[:, kc * 128 : (kc + 1) * 128].T
    wa1_arr = np.zeros((128, 1024), np.float32)
    wb1_arr = np.zeros((128, 1024), np.float32)
    for kc in range(4):
        wa1_arr[:, kc * 256 : (kc + 1) * 256] = WA1[kc * 128 : (kc + 1) * 128, :]
        wb1_arr[:, kc * 256 : (kc + 1) * 256] = WB1[kc * 128 : (kc + 1) * 128, :]
    wa2_arr = np.zeros((128, 2048), np.float32)
    wb2_arr = np.zeros((128, 2048), np.float32)
    for kc in range(2):
        wa2_arr[:, kc * 1024 : (kc + 1) * 1024] = WA2[kc * 128 : (kc + 1) * 128, :]
        wb2_arr[:, kc * 1024 : (kc + 1) * 1024] = WB2[kc * 128 : (kc + 1) * 128, :]
    ba1_arr = np.ascontiguousarray(bA1.reshape(2, 128).T)
    bb1_arr = np.ascontiguousarray(bB1.reshape(2, 128).T)
    ident_arr = np.eye(16, dtype=np.float32)

    in_maps = []
    for c in range(NCORES):
        lt0 = c * LT
        # reorder each layer's 7 projections into stage order s (t = TOF[s])
        wp_full = W_proj[:, lt0 * PED : (lt0 + LT) * PED].reshape(EMB, L, T, PED)
        bp_full = b_proj[lt0 * PED : (lt0 + LT) * PED].reshape(L, T, PED)
        wp_c = np.ascontiguousarray(
            wp_full[:, :, TOF, :].reshape(EMB, LT * PED).astype(NPBF16)
        )
        bias2_row = np.ascontiguousarray(bp_full[:, TOF, :].reshape(L * CHUNK))
        bias2_arr = np.broadcast_to(bias2_row[None, :], (2, L * CHUNK)).copy()
        # rows are b-major stage order: row = b*7 + s, s = RT[t]
        sca_row = np.zeros(2 * LT, np.float32)
        scb_row = np.zeros(2 * LT, np.float32)
        for l in range(L):
            for t in range(T):
                for b in range(2):
                    r = l * RPL + b * T + RT[t]
                    sca_row[r] = scales[lt0 + l * T + t, 0]
                    scb_row[r] = scales[lt0 + l * T + t, 1]
        sca_arr = np.broadcast_to(sca_row[None, :], (128, 2 * LT)).copy()
        scb_arr = np.broadcast_to(scb_row[None, :], (128, 2 * LT)).copy()
        sba2_arr = np.zeros((16, L * 1024), np.float32)
        sbb2_arr = np.zeros((16, L * 1024), np.float32)
        for l in range(L):
            for t in range(T):
                for b in range(2):
                    r = b * T + RT[t]
                    sba2_arr[r, l * 1024 : (l + 1) * 1024] = (
                        scales[lt0 + l * T + t, 0] * bA2
                    )
                    sbb2_arr[r, l * 1024 : (l + 1) * 1024] = (
                        scales[lt0 + l * T + t, 1] * bB2
                    )
        in_maps.append(
            {
                "cond": cond_arr.astype(NPBF16),
                "wp": wp_c,
                "bias2": bias2_arr.astype(NPBF16),
                "wa1": wa1_arr.astype(NPBF16),
                "wb1": wb1_arr.astype(NPBF16),
                "wa2": wa2_arr.astype(NPBF16),
                "wb2": wb2_arr.astype(NPBF16),
                "ba1": ba1_arr,
                "bb1": bb1_arr,
                "sca": sca_arr.astype(NPBF16),
                "scb": scb_arr.astype(NPBF16),
                "sba2": sba2_arr.astype(NPBF16),
                "sbb2": sbb2_arr.astype(NPBF16),
                "ident": ident_arr.astype(NPBF16),
            }
        )
    return in_maps


def _ensure_ntff_hook():
    """Register the axon NTFF profile hook if the boot didn't (module was
    missing at boot time)."""
    import types

    ah = sys.modules.get("antenv.axon_hooks")
    if ah is None:
        ah = types.ModuleType("antenv.axon_hooks")
        ah._hook = None

        def _set(h, _m=ah):
            _m._hook = h

        def _get(_m=ah):
            return _m._hook

        ah.set_axon_ntff_profile_hook = _set
        ah.get_axon_ntff_profile_hook = _get
        sys.modules["antenv.axon_hooks"] = ah
        import antenv

        antenv.axon_hooks = ah
    if ah.get_axon_ntff_profile_hook() is None:
        if "/root/.axon_site" not in sys.path:
            sys.path.insert(0, "/root/.axon_site")
        from trn_agent_boot.trn_boot import _ntff_profile_via_ctypes

        hook = _ntff_profile_via_ctypes("/opt/axon/libaxon_pjrt.so")
        if hook is not None:
            ah.set_axon_ntff_profile_hook(hook)


def _run(inputs, trace=False):
    if trace:
        _ensure_ntff_hook()
    nc = _get_nc()
    in_maps = _marshal(inputs)
    res = run_bass_kernel_spmd(nc, in_maps, list(range(NCORES)), trace=trace)
    full = np.empty((2, NUM_LAYERS, LAYER_SIZE), np.float32)
    for c in range(NCORES):
        full[:, c * L : (c + 1) * L, :] = (
            res.results[c]["out"].astype(np.float32).reshape(2, L, LAYER_SIZE)
        )
    return full.reshape(2, -1), res


def kernel(**inputs) -> np.ndarray:
    out, _ = _run(inputs, trace=False)
    return out
